# revision 21
# baseline (speedup 1.0000x reference)
"""Trainium2 Bass kernel for nn_DRuleLoss.

Math (exact collapse of the reference):
    branches = min(H.sum(1), 1)                 # [n]
    bc       = branches.sum()
    rmax     = H.max(1); rmin = H.min(1)        # [n]
    loss = sum_{b,i} [ branches[i]*p + branches[i]*p*max(p*rmax[i], p*rmin[i]) ] / bc
         (p = y_pred[b,i])

For p >= 0 (graded inputs are uniform [0,1)): max(p*rmax, p*rmin) = p*rmax, so
    loss = sum_i w1[i]*colsum_p[i] + sum_i w2a[i]*colsum_p2[i] + neg_corr
with w1 = branches/bc, w2a = branches*rmax/bc.

H is a tree adjacency (one parent per non-root row), so w1 and w2a are
the CONSTANT 1/bc on every column except a handful of deviants (just
column 0 for the root).  The device therefore computes only the
unweighted scalar  S = sum_{b,i} (p + p^2)  per core; the host forms
    loss = alpha*S_total + sum_{i in D} [(w1[i]-alpha)*colsum_p[i]
                                         + (w2a[i]-alpha)*colsum_p2[i]]
           + sum_i (w2b[i]-w2a[i]) * negsum2[i]
where alpha is the modal weight, D the deviant columns (exact numpy on
the few y_pred[:, D] columns), and the last term the exact correction
for negative p (empty for graded data).  Fully general for any H.

Device strategy (data-parallel, 8 cores, batch-sharded):
  The whole per-element reduction collapses into ScalarE:
      (2p + 1)^2 = 4*(p + p^2) + 1
  so one ACT pass  activation(Square, scale=2, bias=1, accum_out=...)
  per slab computes the per-partition sum of (2p+1)^2 directly -- no
  TensorE, no PSUM, no DVE.  The host undoes the affine exactly
  ((S - count) / 4) in f64.

  The core's [512, 8192] shard is viewed as [128, 32768] (4 contiguous
  DRAM rows per partition -> 128 contiguous 128 KiB runs) and streamed
  in NSLAB column slabs, ALL on the SP HWDGE ring.  Measured on HW:
  one ring alone sustains the full ~351 GB/s HBM-per-NC rate (two
  rings are no faster), and a single ring delivers slabs strictly
  in-order, so exactly one activation remains after the final DMA
  byte lands; with two balanced rings both finish at stream end,
  leaving two full-width tail activations (~+2.5 us/pass).  The slab
  taper (4096-wide bulk, 1024-wide tail) keeps that last activation
  small.  ScalarE consumes slabs in arrival order (~31 us busy, under
  the ~48 us DMA stream), accumulating each slab's per-partition sum
  into acc[:, k].  One scratch buffer, not two: the activation main
  output is write-only garbage, and a second buffer measured 2.5 us
  SLOWER.  The epilogue ships acc [128, NSLAB] (4 KiB); the host
  reduces it in f64 and applies alpha/corr.
"""

import numpy as np

import concourse.tile as tile
import concourse.mybir as mybir
from concourse import bacc
from concourse.bass_utils import run_bass_kernel_spmd

N_CORES = 8
B, N = 4096, 8192
BS = B // N_CORES        # 512 rows per core
P = 128                  # SBUF partitions
FPP = BS * N // P        # 32768 f32 per partition (128 KiB)
F32 = mybir.dt.float32

# --- tunables (module-level so the experiment harness can sweep them) ---
# slab widths in f32-per-partition; must sum to FPP.  Measured-best
# (52.9 us/pass vs 55.4 for two-ring uniform): ONE ring sustains the
# full ~351 GB/s HBM rate, and a single ring delivers slabs strictly
# in-order, so exactly one activation remains after the final DMA byte
# lands -- the taper keeps that one tiny.  Two balanced rings both
# finish at stream end, leaving TWO full-width tail activations.
SLAB_WIDTHS = (4096, 4096, 4096, 4096, 4096, 4096, 3072, 2048,
               1024, 1024, 1024)
NSLAB = len(SLAB_WIDTHS)
# queue per slab: 0 = SP HWDGE (nc.sync), 1 = ACT HWDGE (nc.scalar),
# 2 = SWDGE (nc.gpsimd)
SLAB_QUEUE = (0,) * 11
# compute engine per slab: 0 = ScalarE Square(2x+1) (host affine fixup),
# 1 = DVE (x+1)*x (exact)
SLAB_COMPUTE = (0,) * 11
SLAB_BUFS = 6            # in-flight slab tiles (max-width sized)
SCRATCH_BUFS = 1         # write-only, same-engine WAW only
EMPTY_BODY = False       # experiment harness: measure bare loop overhead
NO_COMPUTE = False       # experiment harness: DMAs only, no activations
ACT_ONLY = False         # experiment harness: activations only, no DMAs
TAIL_MODE = 0            # 1: last act reads slab 0 (probe); 2: act k
                         # reads slab k-1 (probe)
STAGGERED = False        # For_i(staggered_reset=...) in the loop harness

_NC_CACHE = {}
_STATE = {}              # trace-time scratch for experiment modes
LAST_RESULTS = None      # BassKernelResults of the most recent device run


def build_pools(tc):
    import contextlib
    st = contextlib.ExitStack()
    pools = {
        "slabs": st.enter_context(tc.tile_pool(name="slabs", bufs=SLAB_BUFS)),
        "scratch": st.enter_context(
            tc.tile_pool(name="scratch", bufs=SCRATCH_BUFS)),
        "small": st.enter_context(tc.tile_pool(name="small", bufs=1)),
    }
    return st, pools


def build_prelude(nc, pools):
    """One-time setup: the per-(partition, slab) accumulator tile."""
    acc = pools["small"].tile([P, NSLAB], F32)
    if NO_COMPUTE:
        nc.vector.memset(acc[:], 0.0)
    if ACT_ONLY:
        src = pools["small"].tile([P, max(SLAB_WIDTHS)], F32)
        nc.vector.memset(src[:], 0.5)
        _STATE["act_src"] = src
    return acc


def build_body(nc, y, pools, acc):
    """One full pass over the core's [128, FPP] shard view."""
    slabs, scratch = pools["slabs"], pools["scratch"]
    if EMPTY_BODY:
        sc = scratch.tile([P, 512], F32, tag="sc", name="sc")
        nc.vector.memset(sc[:], 1.0)
        nc.scalar.activation(sc[:], sc[:],
                             mybir.ActivationFunctionType.Square,
                             bias=1.0, scale=2.0,
                             accum_out=acc[:, 0:1])
        return
    assert sum(SLAB_WIDTHS) == FPP
    wmax = max(SLAB_WIDTHS)
    engines = (nc.sync, nc.scalar, nc.gpsimd)

    offs = [0]
    for w in SLAB_WIDTHS:
        offs.append(offs[-1] + w)

    if ACT_ONLY:
        src = _STATE["act_src"]
        for k in range(NSLAB):
            w = SLAB_WIDTHS[k]
            sc = scratch.tile([P, wmax], F32, tag="sc", name="sc")
            nc.scalar.activation(sc[:, :w], src[:, :w],
                                 mybir.ActivationFunctionType.Square,
                                 bias=1.0, scale=2.0,
                                 accum_out=acc[:, k:k + 1])
        return

    tiles = {}
    for k in range(NSLAB):
        w = SLAB_WIDTHS[k]
        tl = slabs.tile([P, wmax], F32, tag="slab", name="slab")
        engines[SLAB_QUEUE[k]].dma_start(tl[:, :w], y[:, offs[k]:offs[k] + w])
        tiles[k] = tl

    if NO_COMPUTE:
        return

    for k in range(NSLAB):
        w = SLAB_WIDTHS[k]
        if TAIL_MODE == 1 and k == NSLAB - 1:
            tl = tiles[0]           # timing probe: skip the tail wait
        elif TAIL_MODE == 2:
            tl = tiles[max(k - 1, 0)]  # timing probe: shift deps by one
        else:
            tl = tiles[k]
        sc = scratch.tile([P, wmax], F32, tag="sc", name="sc")
        if SLAB_COMPUTE[k] == 0:
            # (2p+1)^2 = 4*(p + p^2) + 1; host undoes the affine exactly.
            # bias=1.0 rides the pre-registered const AP; scale stays an
            # immediate, so no new const tensors are needed.
            nc.scalar.activation(sc[:, :w], tl[:, :w],
                                 mybir.ActivationFunctionType.Square,
                                 bias=1.0, scale=2.0,
                                 accum_out=acc[:, k:k + 1])
        else:
            # sum((p+1)*p) = sum(p + p^2) exactly
            nc.vector.scalar_tensor_tensor(
                out=sc[:, :w], in0=tl[:, :w], scalar=1.0, in1=tl[:, :w],
                op0=mybir.AluOpType.add, op1=mybir.AluOpType.mult,
                accum_out=acc[:, k:k + 1])


def build_epilogue(nc, out, acc):
    nc.sync.dma_start(out[:], acc[:])


def _build_nc():
    nc = bacc.Bacc("TRN2", target_bir_lowering=False, debug=False,
                   num_devices=N_CORES)
    y = nc.dram_tensor("y", [P, FPP], F32, kind="ExternalInput")
    out = nc.dram_tensor("out", [P, NSLAB], F32, kind="ExternalOutput")

    with tile.TileContext(nc) as tc:
        st, pools = build_pools(tc)
        with st:
            acc = build_prelude(nc, pools)
            build_body(nc, y, pools, acc)
            build_epilogue(nc, out, acc)

    nc.compile()
    return nc


def _get_nc():
    if "nc" not in _NC_CACHE:
        _NC_CACHE["nc"] = _build_nc()
    return _NC_CACHE["nc"]


def kernel(y_pred, H, y_true):
    global LAST_RESULTS
    y_pred = np.ascontiguousarray(np.asarray(y_pred, dtype=np.float32))
    H = np.asarray(H, dtype=np.float32)

    branches = np.minimum(H.sum(axis=1, dtype=np.float64), 1.0)
    bc = float(branches.sum())
    rmax = H.max(axis=1).astype(np.float64)
    rmin = H.min(axis=1).astype(np.float64)
    w1 = (branches / bc).astype(np.float32)
    w2a = (branches * rmax / bc).astype(np.float32)
    w2b = (branches * rmin / bc).astype(np.float32)

    # modal weight: device computes the unweighted sum, host rescales
    vals, counts = np.unique(w1, return_counts=True)
    alpha = float(vals[np.argmax(counts)])
    dev = (w1 != np.float32(alpha)) | (w2a != np.float32(alpha))
    D = np.nonzero(dev)[0]

    corr = 0.0
    if D.size:
        yd = y_pred[:, D].astype(np.float64)
        cp = yd.sum(axis=0)
        cp2 = (yd * yd).sum(axis=0)
        corr += float(((w1[D].astype(np.float64) - alpha) * cp).sum()
                      + ((w2a[D].astype(np.float64) - alpha) * cp2).sum())

    # Device assumes max(p*rmax, p*rmin) == p*rmax, true for p >= 0.
    # Exact correction for any negative p (graded inputs have none).
    if np.any(y_pred < 0):
        neg = np.minimum(y_pred, 0.0).astype(np.float64)
        corr += float(((neg * neg) @ (w2b - w2a).astype(np.float64)).sum())

    nc = _get_nc()
    in_maps = [
        {"y": np.ascontiguousarray(
            y_pred[i * BS:(i + 1) * BS]).reshape(P, FPP)}
        for i in range(N_CORES)
    ]
    LAST_RESULTS = run_bass_kernel_spmd(nc, in_maps,
                                        core_ids=list(range(N_CORES)))
    # ScalarE slab columns hold sum((2p+1)^2) = 4*sum(p+p^2) + count;
    # DVE slab columns hold sum((p+1)*p) = sum(p+p^2) exactly.
    sc_cols = [k for k in range(NSLAB) if SLAB_COMPUTE[k] == 0]
    dv_cols = [k for k in range(NSLAB) if SLAB_COMPUTE[k] == 1]
    s_scalar = sum(
        float(r["out"][:, sc_cols].sum(dtype=np.float64))
        for r in LAST_RESULTS.results
    )
    s_dve = sum(
        float(r["out"][:, dv_cols].sum(dtype=np.float64))
        for r in LAST_RESULTS.results
    )
    n_scalar_elems = N_CORES * P * sum(SLAB_WIDTHS[k] for k in sc_cols)
    total = (s_scalar - n_scalar_elems) / 4.0 + s_dve
    return np.float32(alpha * total + corr)
